# revision 9
# baseline (speedup 1.0000x reference)
"""Trainium2 Bass kernel for 3-iteration GMM-EM slot attention.

B=8, N=4096, K=16, D=128. Data-parallel over B across 8 NeuronCores
(each core runs EM for one batch element; no collectives).

Math per core, per EM iteration:
  E: logits[n,k] = -0.5*sum_d (x-mu)^2/(sig+EPS) - 0.5*sum_d log(2pi sig+EPS) + log(pi+EPS)
     = xsq @ W1 + x @ A + c   with W1 = -0.5/(sig+EPS), A = mu/(sig+EPS),
       c = sum_d(-0.5 mu^2/(sig+EPS) - 0.5 log(2pi sig+EPS)) + log(pi+EPS)
     gamma = softmax_k(logits)
  M: S = gamma^T @ [x | x^2 | 1]  (K x 2D+1)
     gs = S[:,2D]+EPS; pi = S[:,2D]/N; mu = S[:,:D]/gs; sigma = S[:,D:2D]/gs - mu^2
Output: slots = mu + sigma*noise.

Matmul inputs are bf16 (fp32 PSUM accumulate); the softmax constant c and all
inter-step math stay fp32. Validated vs the fp32 reference: norm rel err ~4e-3.
"""
import sys
import numpy as np
import ml_dtypes

if '/opt/trn_rl_repo' not in sys.path:
    sys.path.insert(0, '/opt/trn_rl_repo')

import concourse.bass as bass
import concourse.tile as tile
from concourse import bacc, mybir
from concourse.bass_utils import run_bass_kernel_spmd
from concourse.masks import make_identity

# All activation funcs we use (exp, ln, copy, square, identity) live in the
# 'natural_log_exp_and_others' ACT table set.  The default set chooser picks
# per-function-first-match, which alternates table sets between Exp and Ln and
# pays ~2.7us per ACT_TABLE_LOAD.  Strip the shared funcs from all other sets
# so the chooser is forced onto the one set and emits a single load.
_KEEP_SET = 'natural_log_exp_and_others'
_orig_gat = bacc.get_activation_tables


def _patched_gat(arch):
    t = _orig_gat(arch)
    if _KEEP_SET in t:
        shared = t[_KEEP_SET]
        t = {name: (funcs if name == _KEEP_SET else funcs - shared)
             for name, funcs in t.items()}
    return t


bacc.get_activation_tables = _patched_gat

F32 = mybir.dt.float32
BF16 = mybir.dt.bfloat16
AF = mybir.ActivationFunctionType
ALU = mybir.AluOpType
AX = mybir.AxisListType

B, N, K, D = 8, 4096, 16, 128
NT = N // 128          # 32 token tiles
GRP = 8                # tiles per softmax batch
NG = NT // GRP         # 4 groups
NUM_ITER = 3
EPS = 1e-8
XW = 2 * D + 2         # xcat row: [x(128) | x^2(128) | 1 | pad]


def _build():
    nc = bacc.Bacc('TRN2', target_bir_lowering=False, debug=False, num_devices=8)

    x_tm_ext = nc.dram_tensor('x_tm', [N, D], BF16, kind='ExternalInput').ap()
    x_fm_ext = nc.dram_tensor('x_fm', [D, N], BF16, kind='ExternalInput').ap()
    mu0_ext = nc.dram_tensor('mu0', [K, D], F32, kind='ExternalInput').ap()
    ls0_ext = nc.dram_tensor('logsig0', [K, D], F32, kind='ExternalInput').ap()
    pi0_ext = nc.dram_tensor('pi0', [K, 1], F32, kind='ExternalInput').ap()
    noise_ext = nc.dram_tensor('noise', [K, D], F32, kind='ExternalInput').ap()
    out_ext = nc.dram_tensor('out', [K, D], F32, kind='ExternalOutput').ap()

    with tile.TileContext(nc) as tc:
        with tc.tile_pool(name='const', bufs=1) as constp, \
             tc.tile_pool(name='work', bufs=2) as workp, \
             tc.tile_pool(name='pslog', bufs=4, space='PSUM') as pslog, \
             tc.tile_pool(name='psS', bufs=1, space='PSUM') as psS, \
             tc.tile_pool(name='psT', bufs=1, space='PSUM') as psT, \
             tc.tile_pool(name='psR', bufs=1, space='PSUM') as psR:

            ident = constp.tile([16, 16], F32)
            make_identity(nc, ident[:])
            ones_row = constp.tile([1, 128], F32)
            nc.vector.memset(ones_row[:], 1.0)
            eps_col = constp.tile([K, 1], F32)
            nc.vector.memset(eps_col[:], EPS)

            # ---- load x in both layouts; build xcat = [x | x^2 | 1 | pad] token-major
            xcat = constp.tile([128, NT, XW], BF16)
            nc.sync.dma_start(xcat[:, :, 0:D], x_tm_ext.rearrange("(t p) d -> p t d", p=128))
            nc.scalar.activation(xcat[:, :, D:2 * D], xcat[:, :, 0:D], AF.Square)
            nc.vector.memset(xcat[:, :, 2 * D:2 * D + 1], 1.0)

            x_fm = constp.tile([128, N], BF16)
            nc.sync.dma_start(x_fm[:], x_fm_ext)
            xsq_fm = constp.tile([128, N], BF16)
            nc.vector.tensor_mul(xsq_fm[:], x_fm[:], x_fm[:])

            noise_sb = constp.tile([K, D], F32)
            nc.sync.dma_start(noise_sb[:], noise_ext)

            # ---- initial slot-major stats
            mu_s = workp.tile([K, D], F32, tag='mu_s')
            nc.sync.dma_start(mu_s[:], mu0_ext)
            sigma_s = workp.tile([K, D], F32, tag='sigma_s')
            ls_sb = workp.tile([K, D], F32, tag='ls')
            nc.sync.dma_start(ls_sb[:], ls0_ext)
            nc.scalar.activation(sigma_s[:], ls_sb[:], AF.Exp)
            pi_s = workp.tile([K, 1], F32, tag='pi_s')
            nc.sync.dma_start(pi_s[:], pi0_ext)

            def make_weights(mu_s, sigma_s, pi_s):
                """slot-major [K,*] fp32 stats -> (W1_fm bf16, A_fm bf16, c_bcast f32)."""
                # lg = log(sigma+EPS) with accum c2raw = sum_d lg; isig = exp(-lg)
                # log(2pi*sigma+EPS) == log(2pi) + log(sigma+EPS/2pi) ~= log(2pi) + lg
                # (sigma >> EPS always), so sum_d log(2pi sig+EPS) = c2raw + D*log(2pi).
                lg = workp.tile([K, D], F32, tag='lg')
                c2raw = workp.tile([K, 1], F32, tag='c2')
                nc.scalar.activation(lg[:], sigma_s[:], AF.Ln, bias=eps_col[:],
                                     accum_out=c2raw[:])
                isig = workp.tile([K, D], F32, tag='isig')
                nc.scalar.activation(isig[:], lg[:], AF.Exp, scale=-1.0)
                # mu2 = mu^2 ; c1 = sum_d mu2*isig
                mu2 = workp.tile([K, D], F32, tag='mu2')
                nc.vector.tensor_mul(mu2[:], mu_s[:], mu_s[:])
                junk = workp.tile([K, D], F32, tag='junk')
                c1 = workp.tile([K, 1], F32, tag='c1')
                nc.vector.tensor_mul(junk[:], mu2[:], isig[:])
                nc.vector.tensor_reduce(c1[:], junk[:], axis=AX.X, op=ALU.add)
                # logpi = log(pi+EPS); c = -0.5*(c1 + c2raw + D*log(2pi)) + logpi
                logpi = workp.tile([K, 1], F32, tag='logpi')
                nc.scalar.activation(logpi[:], pi_s[:], AF.Ln, bias=eps_col[:])
                c12 = workp.tile([K, 1], F32, tag='c12')
                nc.vector.tensor_add(c12[:], c1[:], c2raw[:])
                c_s = workp.tile([K, 1], F32, tag='c_s')
                nc.vector.tensor_scalar(c_s[:], c12[:], float(D * np.log(2.0 * np.pi)),
                                        -0.5, ALU.add, ALU.mult)
                nc.vector.tensor_add(c_s[:], c_s[:], logpi[:])
                # W1 = -0.5*isig ; A = mu*isig  (slot-major f32)
                w1_s = workp.tile([K, D], F32, tag='w1_s')
                nc.scalar.mul(w1_s[:], isig[:], -0.5)
                a_s = workp.tile([K, D], F32, tag='a_s')
                nc.vector.tensor_mul(a_s[:], mu_s[:], isig[:])
                # transposes to feature-major, cast to bf16
                w1p = psT.tile([D, K], F32, tag='wT')
                nc.tensor.transpose(w1p[:], w1_s[:], ident[:])
                W1_fm = workp.tile([D, K], BF16, tag='W1_fm')
                nc.scalar.copy(W1_fm[:], w1p[:])
                ap = psT.tile([D, K], F32, tag='wT')
                nc.tensor.transpose(ap[:], a_s[:], ident[:])
                A_fm = workp.tile([D, K], BF16, tag='A_fm')
                nc.scalar.copy(A_fm[:], ap[:])
                # c row + broadcast to all 128 partitions via ones-matmul
                cp = psT.tile([1, K], F32, tag='cT')
                nc.tensor.transpose(cp[:], c_s[:], ident[:])
                c_row = workp.tile([1, K], F32, tag='c_row')
                nc.vector.tensor_copy(c_row[:], cp[:])
                cbp = psR.tile([128, K], F32, tag='cb')
                nc.tensor.matmul(cbp[:], ones_row[:], c_row[:], start=True, stop=True)
                c_bcast = workp.tile([128, K], F32, tag='c_bcast')
                nc.scalar.copy(c_bcast[:], cbp[:])
                return W1_fm, A_fm, c_bcast

            W1_fm, A_fm, c_bcast = make_weights(mu_s, sigma_s, pi_s)

            for it in range(NUM_ITER):
                # ---- E-step + softmax, 4 groups of 8 tiles
                gbf = workp.tile([128, NT, K], BF16, tag='gbf')
                for g in range(NG):
                    lp = pslog.tile([128, GRP * K], F32, tag='lp')
                    for j in range(GRP):
                        t = g * GRP + j
                        sl = lp[:, j * K:(j + 1) * K]
                        nc.tensor.matmul(sl, xsq_fm[:, t * 128:(t + 1) * 128], W1_fm[:],
                                         start=True, stop=False)
                        nc.tensor.matmul(sl, x_fm[:, t * 128:(t + 1) * 128], A_fm[:],
                                         start=False, stop=True)
                    l_sb = workp.tile([128, GRP, K], F32, tag='l_sb')
                    nc.vector.tensor_add(l_sb[:], lp[:].rearrange("p (t k) -> p t k", k=K),
                                         c_bcast[:].unsqueeze(1).broadcast_to([128, GRP, K]))
                    negm = workp.tile([128, GRP], F32, tag='negm')
                    nc.vector.tensor_reduce(negm[:], l_sb[:], axis=AX.X, op=ALU.max, negate=True)
                    lsub = workp.tile([128, GRP, K], F32, tag='lsub')
                    nc.vector.tensor_add(lsub[:], l_sb[:],
                                         negm[:].unsqueeze(2).broadcast_to([128, GRP, K]))
                    gam = workp.tile([128, GRP, K], F32, tag='gam')
                    nc.scalar.activation(gam[:], lsub[:], AF.Exp)
                    zsum = workp.tile([128, GRP], F32, tag='zsum')
                    nc.vector.tensor_reduce(zsum[:], gam[:], axis=AX.X, op=ALU.add)
                    rz = workp.tile([128, GRP], F32, tag='rz')
                    nc.vector.reciprocal(rz[:], zsum[:])
                    nc.vector.tensor_mul(gbf[:, g * GRP:(g + 1) * GRP, :], gam[:],
                                         rz[:].unsqueeze(2).broadcast_to([128, GRP, K]))

                # ---- M-step: S = gamma^T @ [x | x^2 | 1]
                sp = psS.tile([K, 2 * D + 1], F32, tag='sp')
                for t in range(NT):
                    nc.tensor.matmul(sp[:], gbf[:, t, :], xcat[:, t, 0:2 * D + 1],
                                     start=(t == 0), stop=(t == NT - 1))
                s_sb = workp.tile([K, 2 * D + 1], F32, tag='s_sb')
                nc.scalar.copy(s_sb[:], sp[:])

                # gs = S[:,2D]+EPS ; rgs = 1/gs ; pi = S[:,2D]/N
                rgs = workp.tile([K, 1], F32, tag='rgs')
                gs = workp.tile([K, 1], F32, tag='gs')
                nc.vector.tensor_scalar(gs[:], s_sb[:, 2 * D:2 * D + 1], EPS, None, ALU.add)
                nc.vector.reciprocal(rgs[:], gs[:])
                pi_s = workp.tile([K, 1], F32, tag='pi_s')
                nc.scalar.mul(pi_s[:], s_sb[:, 2 * D:2 * D + 1], 1.0 / N)
                # mu = S0*rgs ; ex2 = S1*rgs ; sigma = ex2 - mu^2
                mu_s = workp.tile([K, D], F32, tag='mu_s')
                nc.vector.tensor_mul(mu_s[:], s_sb[:, 0:D],
                                     rgs[:].broadcast_to([K, D]))
                ex2 = workp.tile([K, D], F32, tag='ex2')
                nc.vector.tensor_mul(ex2[:], s_sb[:, D:2 * D],
                                     rgs[:].broadcast_to([K, D]))
                mu2b = workp.tile([K, D], F32, tag='mu2b')
                nc.vector.tensor_mul(mu2b[:], mu_s[:], mu_s[:])
                sigma_s = workp.tile([K, D], F32, tag='sigma_s')
                nc.vector.tensor_sub(sigma_s[:], ex2[:], mu2b[:])

                if it < NUM_ITER - 1:
                    W1_fm, A_fm, c_bcast = make_weights(mu_s, sigma_s, pi_s)
                else:
                    # slots = mu + sigma*noise
                    slots = workp.tile([K, D], F32, tag='slots')
                    nc.vector.tensor_mul(slots[:], sigma_s[:], noise_sb[:])
                    nc.vector.tensor_add(slots[:], slots[:], mu_s[:])
                    nc.sync.dma_start(out_ext, slots[:])

    nc.compile()
    return nc


_NC = None


def _get_nc():
    global _NC
    if _NC is None:
        _NC = _build()
    return _NC


def kernel(inputs, slots_mu, slots_log_sigma, mixing_coefficients, noise):
    nc = _get_nc()
    inputs = np.asarray(inputs, dtype=np.float32)
    mu0 = np.ascontiguousarray(np.asarray(slots_mu, dtype=np.float32)[0])        # (K,D)
    ls0 = np.ascontiguousarray(np.asarray(slots_log_sigma, dtype=np.float32)[0])  # (K,D)
    pi0 = np.ascontiguousarray(np.asarray(mixing_coefficients, dtype=np.float32)[0].reshape(K, 1))
    noise = np.asarray(noise, dtype=np.float32)                                  # (B,K,D)

    in_maps = []
    for b in range(B):
        x_bf = inputs[b].astype(ml_dtypes.bfloat16)          # (N,D)
        in_maps.append({
            'x_tm': x_bf,
            'x_fm': np.ascontiguousarray(x_bf.T),
            'mu0': mu0,
            'logsig0': ls0,
            'pi0': pi0,
            'noise': np.ascontiguousarray(noise[b]),
        })

    res = run_bass_kernel_spmd(nc, in_maps, core_ids=list(range(B)))
    return np.stack([res.results[i]['out'] for i in range(B)]).astype(np.float32)


if __name__ == '__main__':
    rng = np.random.default_rng(0)
    ins = {
        'inputs': rng.standard_normal((B, N, D)).astype(np.float32),
        'slots_mu': rng.standard_normal((1, K, D)).astype(np.float32),
        'slots_log_sigma': rng.standard_normal((1, K, D)).astype(np.float32),
        'mixing_coefficients': np.full((1, K), 1.0 / K, dtype=np.float32),
        'noise': rng.standard_normal((B, K, D)).astype(np.float32),
    }
    out = kernel(**ins)
    print('kernel ran, out shape', out.shape, 'finite:', np.isfinite(out).all())

